# revision 15
# baseline (speedup 1.0000x reference)
"""Trainium2 Bass kernel for nn_Conv1dAttention.

Math (per sample):
  q,k,v,pe = lrelu(bn(conv1d(x, W_p)))           # [C=128, L=2048], Cin=64, K=3
  S = q^T k                                      # [L, L]
  P = softmax_rows(S)                            # softmax over last axis
  out = v @ P + pe                               # [C, L]

Sharding: data-parallel over batch B=16 across 8 NeuronCores (2 samples/core).
Same NEFF on all cores, per-core input shards, no collectives.

Design notes (v2 — engine-balance rewrite of the 142us baseline):
  - BN folded into conv weights/bias on host; bias via ones-row in im2col.
  - Conv contraction 192 = 128-row chunk (k=0,k=1) + 65-row chunk (k=2+bias).
  - Q,K,PE in [c,l] layout; V directly transposed [l,c] to feed V@P.
  - All matmul operands bf16; PSUM accumulation fp32.
  - LReLU drain of a [128,512] conv quarter is forcibly 2 ops (HW: only one
    PSUM operand per vector op): a psum->sbuf bf16 copy (ACT activation-Copy
    or DVE tensor_scalar) + a DVE stt max(0.3y, y). Placement is tuned so
    ACT(exp-bound) and DVE finish together.
  - Softmax without max subtraction (logits bounded for this weight scale).
    exp on ACT; row-sums Z either ride the ACT activation accumulator
    (187ns/read aux) or a DVE tensor_tensor_reduce over the bf16 P tile —
    choice per block balances ACT vs DVE.
  - pe is accumulated into the PSUM output accumulator by an identity-
    stationary matmul (PE is cheap), so the finish step is a plain
    psum->sbuf bf16 copy; output DMA'd as bf16 and upcast on host.
  - PSUM: 4 banks out accumulator [128,2048] fp32; 4 banks = two rotating
    [128,1024] tiles shared by S-halves and conv quarters.
  - PE p-state: dummy warm-up matmuls cover the input-DMA latency so real
    convs start at full clock; conv/vt work is dripped into the attention
    stream so the PE queue never drains.
"""

import sys

if "/opt/trn_rl_repo" not in sys.path:
    sys.path.insert(0, "/opt/trn_rl_repo")

from contextlib import ExitStack

import ml_dtypes
import numpy as np

import concourse.bass as bass
import concourse.tile as tile
from concourse import bacc, mybir
from concourse.bass_utils import run_bass_kernel_spmd

B, CIN, COUT, KW, L = 16, 64, 128, 3, 2048
NCORES = 8
BP = B // NCORES  # samples per core
EPS = 1e-5
SLOPE = 0.3
F32 = mybir.dt.float32
BF16 = mybir.dt.bfloat16
NB = L // 128  # 16 l-blocks per sample
HALF = 1024

# ---- schedule knobs -------------------------------------------------------
WARMUP_MM = 92  # dummy 128-free matmuls covering input-DMA latency
# Z placement per global block (s0: 0..15, s1: 16..31): True -> ACT accum,
# False -> DVE tensor_tensor_reduce.  ACT is exp-bound, so only ~25 fit.
Z_ON_ACT = [True] * 16 + [True, False] * 8  # alternate s1 Z: balance ACT/DVE
# drain-copy placement: quarters handled by ACT (activation Copy); the rest
# go to DVE.  Keyed by (sample, proj, quarter) at emission sites below.
# ---------------------------------------------------------------------------

_CACHE = {}


def _body(ctx, tc, x, w1, w2, zc, onesrow, out):
    nc = tc.nc
    amax = mybir.AluOpType.max
    mult = mybir.AluOpType.mult
    add = mybir.AluOpType.add
    Exp = mybir.ActivationFunctionType.Exp
    Copy = mybir.ActivationFunctionType.Copy

    wpool = ctx.enter_context(tc.tile_pool(name="wpool", bufs=1))
    xpool = ctx.enter_context(tc.tile_pool(name="xpool", bufs=2))
    apool = ctx.enter_context(tc.tile_pool(name="apool", bufs=2))
    ppool = ctx.enter_context(tc.tile_pool(name="ppool", bufs=5))
    opool = ctx.enter_context(tc.tile_pool(name="opool", bufs=2))
    vpool = ctx.enter_context(tc.tile_pool(name="vpool", bufs=4))
    zpool = ctx.enter_context(tc.tile_pool(name="zpool", bufs=4))
    lpool = ctx.enter_context(tc.tile_pool(name="lpool", bufs=3))
    psA = ctx.enter_context(tc.tile_pool(name="psA", bufs=2, space="PSUM"))
    psO = ctx.enter_context(tc.tile_pool(name="psO", bufs=1, space="PSUM"))

    # --- weights + constants -------------------------------------------------
    # All conv weights + the identity ride in TWO packed DMAs on the ACT
    # queue (idle until the first exp); the SP queue is reserved for x so the
    # critical-path im2col tiles are not delayed behind weights.
    # w1 layout: [128, 640] = w1_q|w1_k|w1_v|w1_p|ident, w2: [65, 512].
    w1pack = wpool.tile([128, 5 * COUT], BF16, tag="w1pack", name="w1pack")
    nc.scalar.dma_start(out=w1pack[:, :], in_=w1[:, :])
    w2pack = wpool.tile([CIN + 1, 4 * COUT], BF16, tag="w2pack", name="w2pack")
    nc.scalar.dma_start(out=w2pack[:, :], in_=w2[:, :])
    _order = {"q": 0, "k": 1, "v": 2, "p": 3}
    w1_t = {p: w1pack[:, i * COUT : (i + 1) * COUT] for p, i in _order.items()}
    w2_t = {p: w2pack[:, i * COUT : (i + 1) * COUT] for p, i in _order.items()}
    id_t = w1pack[:, 4 * COUT : 5 * COUT]

    def emit_xs(s, prelude):
        # im2col tiles.
        # xs1 rows 0-63  = x[cin, l-1]  (k=0), rows 64-127 = x[cin, l] (k=1)
        # xs2 rows 0-63  = x[cin, l+1]  (k=2), row 64 = ones (bias)
        # prelude: xs2 rides the ACT queue so SP only serializes xs1.
        e2 = nc.scalar if prelude else nc.sync
        xs1 = xpool.tile([128, L], BF16, tag="xs1", name="xs1")
        xs2 = xpool.tile([CIN + 1, L], BF16, tag="xs2", name="xs2")
        nc.sync.dma_start(out=xs1[0:CIN, 0:1], in_=zc[:, :])
        nc.sync.dma_start(out=xs1[0:CIN, 1:L], in_=x[s, :, 0 : L - 1])
        nc.sync.dma_start(out=xs1[CIN:128, 0:L], in_=x[s, :, :])
        e2.dma_start(out=xs2[0:CIN, 0 : L - 1], in_=x[s, :, 1:L])
        e2.dma_start(out=xs2[0:CIN, L - 1 : L], in_=zc[:, :])
        e2.dma_start(out=xs2[CIN : CIN + 1, :], in_=onesrow[:, :])
        return xs1, xs2

    def lrelu_drain(dst_ap, ps_ap, copy_eng):
        # lrelu(y) = max(y, 0.3y).  HW allows only one PSUM operand per
        # vector op, so: copy psum->bf16 (ACT or DVE), then all-SBUF stt.
        w = ps_ap.free_size()
        if copy_eng == "fp":
            # fp32 two-pass for the pe conv (feeds the output directly)
            lt = lpool.tile([128, w], F32, tag="lt", name="lt")
            nc.vector.tensor_scalar_mul(lt[:, :], ps_ap, SLOPE)
            nc.vector.tensor_tensor(dst_ap, ps_ap, lt[:, :], amax)
            return
        yb = lpool.tile([128, w], BF16, tag=f"yb{w}", name="yb")
        if copy_eng == "act":
            nc.scalar.activation(yb[:, :], ps_ap, Copy)
        else:
            nc.vector.tensor_scalar_mul(yb[:, :], ps_ap, 1.0)
        nc.vector.scalar_tensor_tensor(
            dst_ap, yb[:, :], SLOPE, yb[:, :], op0=mult, op1=amax
        )

    def conv_q(xs1, xs2, p, dst, q, copy_eng):
        # one [128,512] quarter of a [c, l]-layout conv
        cps = psA.tile([128, 512], F32, tag="ps", name="cps")
        c0 = q * 512
        nc.tensor.matmul(
            cps[:, :], w1_t[p], xs1[:, c0 : c0 + 512], start=True, stop=False
        )
        nc.tensor.matmul(
            cps[:, :], w2_t[p], xs2[:, c0 : c0 + 512], start=False, stop=True
        )
        lrelu_drain(dst[:, c0 : c0 + 512], cps[:, :], copy_eng)

    def vt_qgroup(xs1, xs2, vt, gh, copy_eng):
        # 4 l-blocks of V in transposed [l, c] layout -> one [128,512] tile
        vps = psA.tile([128, 512], F32, tag="ps", name="vps")
        for i in range(4):
            blk = gh * 4 + i
            lsl = slice(blk * 128, blk * 128 + 128)
            pc = slice(i * 128, i * 128 + 128)
            nc.tensor.matmul(
                vps[:, pc], xs1[:, lsl], w1_t["v"], start=True, stop=False
            )
            nc.tensor.matmul(
                vps[:, pc], xs2[:, lsl], w2_t["v"], start=False, stop=True
            )
        lrelu_drain(vt[:, gh * 512 : (gh + 1) * 512], vps[:, :], copy_eng)

    def make_tiles():
        q_t = apool.tile([128, L], BF16, tag="actq", name="actq")
        k_t = apool.tile([128, L], BF16, tag="actk", name="actk")
        pe_t = apool.tile([128, L], F32, tag="actp", name="actp")
        vt = apool.tile([128, L], BF16, tag="vt", name="vt")
        return q_t, k_t, pe_t, vt

    def attn_body(tiles, blk, z_on_act):
        """S matmuls + exp + normalization prep for one 128-row block."""
        q_t, k_t, pe_t, vt = tiles
        pblk = ppool.tile([128, L], BF16, tag="pblk", name="pblk")
        zz = zpool.tile([128, 2], F32, tag="zz", name="zz")
        for h in range(2):
            sps = psA.tile([128, HALF], F32, tag="ps", name="sps")
            for n in range(2):
                c0 = h * HALF + n * 512
                nc.tensor.matmul(
                    sps[:, n * 512 : n * 512 + 512],
                    q_t[:, blk * 128 : blk * 128 + 128],
                    k_t[:, c0 : c0 + 512],
                    start=True,
                    stop=True,
                )
            nc.scalar.activation(
                pblk[:, h * HALF : (h + 1) * HALF],
                sps[:, :],
                Exp,
                accum_out=zz[:, h : h + 1] if z_on_act else None,
            )
        z = zpool.tile([128, 1], F32, tag="z", name="z")
        if z_on_act:
            nc.vector.tensor_tensor(
                z[:, :], zz[:, 0:1], zz[:, 1:2], add
            )
        else:
            zscr = lpool.tile([128, HALF], BF16, tag="zscr", name="zscr")
            nc.vector.scalar_tensor_tensor(
                zscr[:, :],
                pblk[:, 0:HALF],
                1.0,
                pblk[:, HALF:L],
                op0=mult,
                op1=add,
                accum_out=z[:, :],
            )
        r = zpool.tile([128, 1], F32, tag="r", name="r")
        nc.vector.reciprocal(r[:, :], z[:, :])
        vts = vpool.tile([128, 128], BF16, tag="vts", name="vts")
        nc.vector.tensor_scalar_mul(
            vts[:, :], vt[:, blk * 128 : blk * 128 + 128], r[:, :]
        )
        return pblk, vts

    def out_mms(out_ps, pblk, vts, blk):
        for n in range(4):
            nc.tensor.matmul(
                out_ps[:, n * 512 : n * 512 + 512],
                vts[:, :],
                pblk[:, n * 512 : n * 512 + 512],
                start=(blk == 0),
                stop=(blk == NB - 1),
            )

    def finish_sample(tiles, out_ps, s):
        pe_t = tiles[2]
        outs = opool.tile([128, L], F32, tag="outs", name="outs")
        for h in range(2):
            cols = slice(h * HALF, (h + 1) * HALF)
            nc.vector.tensor_tensor(
                outs[:, cols], out_ps[:, cols], pe_t[:, cols], add
            )
            nc.sync.dma_start(out=out[s, :, cols], in_=outs[:, cols])

    def attention_phase(tiles, out_ps, queue, zofs):
        """Software-pipelined: block b's S/exp runs ahead of block b-1's
        out-matmuls so the PE always feeds ScalarE first. `queue` is a list
        of (deadline_blk, thunk) conv units dripped in as filler."""
        qi = 0
        pending = []
        for blk in range(NB):
            while qi < len(queue) and queue[qi][0] <= blk:
                queue[qi][1]()
                qi += 1
            pblk, vts = attn_body(tiles, blk, Z_ON_ACT[zofs + blk])
            pending.append((pblk, vts, blk))
            # trail the out-matmuls TWO blocks behind the S/exp stream so the
            # exp->Z->recip->vts chain has a full extra block-cycle of slack
            if len(pending) > 2:
                out_mms(out_ps, *pending.pop(0))
            # pace the remaining filler ~evenly over the phase, but let the
            # first blocks run clean so the exp stream primes without psA
            # slots being held by conv drains
            while qi < len(queue) and blk >= 4 and (qi + 1) * (NB - 6) <= (blk - 4) * len(queue):
                queue[qi][1]()
                qi += 1
        while qi < len(queue):
            queue[qi][1]()
            qi += 1
        for p in pending:
            out_mms(out_ps, *p)

    assert BP == 2
    # PE warm-up: dummy matmuls on a memset tile (no DMA dependency) keep the
    # PE pipeline saturated until the first real conv's inputs land, so the
    # p-state ramp reaches full clock before real work.
    wseed = wpool.tile([128, 128], BF16, tag="wseed", name="wseed")
    nc.gpsimd.memset(wseed[:, :], 0.001)
    # table pre-load: a dummy exp as ACT's first instruction pulls the
    # activation-table load into the DMA window.
    scr1 = zpool.tile([128, 1], F32, tag="scr1", name="scr1")
    nc.scalar.activation(scr1[:, :], wseed[:, 0:1], Exp)
    wps = psA.tile([128, 128], F32, tag="ps", name="wps")
    for _ in range(WARMUP_MM):
        nc.tensor.matmul(
            wps[:, :], wseed[:, :], wseed[:, :], start=True, stop=True
        )
    # Prelude: only what attention block 0 needs — all of K, Q quarter 0
    # (covers blocks 0-3), first V^T quarter.  Drain copies ride ACT (idle
    # until the first exp).
    xs0 = emit_xs(0, prelude=True)
    tiles0 = make_tiles()
    q0, k0, pe0, vt0 = tiles0
    for q in range(4):
        conv_q(*xs0, "k", k0, q, "act")
    conv_q(*xs0, "q", q0, 0, "dve")
    vt_qgroup(*xs0, vt0, 0, "dve")
    # Phase B: sample-0 attention with remaining conv work dripped in.
    # Deadlines: vt0 quarter g is read by attn_body(4g); q0 quarter q by
    # attn_body(4q); sample-1 tensors only in phase C; pe0 at finish.
    xs1_ = emit_xs(1, prelude=False)
    tiles1 = make_tiles()
    q1, k1, pe1, vt1 = tiles1
    queueB = [
        (3, lambda: conv_q(*xs0, "q", q0, 1, "act")),
        (3, lambda: vt_qgroup(*xs0, vt0, 1, "dve")),
        (7, lambda: conv_q(*xs0, "q", q0, 2, "dve")),
        (7, lambda: vt_qgroup(*xs0, vt0, 2, "dve")),
        (11, lambda: conv_q(*xs0, "q", q0, 3, "dve")),
        (11, lambda: vt_qgroup(*xs0, vt0, 3, "dve")),
    ]
    for q in range(4):
        queueB.append((99, lambda q=q: conv_q(*xs0, "p", pe0, q, "fp")))
    for q in range(4):
        queueB.append((99, lambda q=q: conv_q(*xs1_, "q", q1, q, "dve")))
    for q in range(4):
        queueB.append((99, lambda q=q: conv_q(*xs1_, "k", k1, q, "dve")))
    queueC = [
        (0, lambda: vt_qgroup(*xs1_, vt1, 0, "dve")),
        (2, lambda: vt_qgroup(*xs1_, vt1, 1, "dve")),
        (5, lambda: vt_qgroup(*xs1_, vt1, 2, "dve")),
        (8, lambda: vt_qgroup(*xs1_, vt1, 3, "dve")),
        (99, lambda: conv_q(*xs1_, "p", pe1, 0, "fp")),
        (99, lambda: conv_q(*xs1_, "p", pe1, 1, "fp")),
        (99, lambda: conv_q(*xs1_, "p", pe1, 2, "fp")),
        (99, lambda: conv_q(*xs1_, "p", pe1, 3, "fp")),
    ]
    out_ps0 = psO.tile([128, L], F32, tag="ops", name="out_ps0")
    attention_phase(tiles0, out_ps0, queueB, zofs=0)
    finish_sample(tiles0, out_ps0, 0)
    out_ps1 = psO.tile([128, L], F32, tag="ops", name="out_ps1")
    attention_phase(tiles1, out_ps1, queueC, zofs=NB)
    finish_sample(tiles1, out_ps1, 1)


def build():
    nc = bacc.Bacc("TRN2", target_bir_lowering=False, debug=False)
    x_d = nc.dram_tensor("x", [BP, CIN, L], BF16, kind="ExternalInput")
    w1_d = nc.dram_tensor("w1pack", [128, 5 * COUT], BF16, kind="ExternalInput")
    w2_d = nc.dram_tensor("w2pack", [CIN + 1, 4 * COUT], BF16, kind="ExternalInput")
    zc_d = nc.dram_tensor("zc", [CIN, 1], BF16, kind="ExternalInput")
    ones_d = nc.dram_tensor("onesrow", [1, L], BF16, kind="ExternalInput")
    out_d = nc.dram_tensor("out", [BP, COUT, L], F32, kind="ExternalOutput")

    with tile.TileContext(nc) as tc, ExitStack() as ctx:
        _body(
            ctx,
            tc,
            x_d.ap(),
            w1_d.ap(),
            w2_d.ap(),
            zc_d.ap(),
            ones_d.ap(),
            out_d.ap(),
        )
    nc.compile()
    return nc


def _fold_weights(w, b, gamma, beta, mean, var):
    """Fold BN affine (fixed mean/var) into conv weights; return im2col chunks."""
    w = np.asarray(w, np.float64)
    scale = np.asarray(gamma, np.float64) / np.sqrt(np.asarray(var, np.float64) + EPS)
    shift = np.asarray(beta, np.float64) - np.asarray(mean, np.float64) * scale
    wf = w * scale[:, None, None]  # [COUT, CIN, K]
    bf = np.asarray(b, np.float64) * scale + shift
    w1 = np.empty((128, COUT), np.float32)
    w1[0:CIN] = wf[:, :, 0].T
    w1[CIN:128] = wf[:, :, 1].T
    w2 = np.empty((CIN + 1, COUT), np.float32)
    w2[0:CIN] = wf[:, :, 2].T
    w2[CIN] = bf
    return w1, w2


def _get_nc():
    if "nc" not in _CACHE:
        _CACHE["nc"] = build()
    return _CACHE["nc"]


def make_in_maps(inputs):
    bf = ml_dtypes.bfloat16
    x = np.ascontiguousarray(np.asarray(inputs["x"], np.float32).astype(bf))
    folded = {}
    for p in "qkvp":
        key = p if p != "p" else "pe"
        folded[p] = _fold_weights(
            inputs[f"{key}_w"],
            inputs[f"{key}_b"],
            inputs[f"{key}_gamma"],
            inputs[f"{key}_beta"],
            inputs[f"{key}_mean"],
            inputs[f"{key}_var"],
        )
    order = "qkvp"
    w1pack = np.empty((128, 5 * COUT), np.float32)
    w2pack = np.empty((CIN + 1, 4 * COUT), np.float32)
    for i, p in enumerate(order):
        w1pack[:, i * COUT : (i + 1) * COUT] = folded[p][0]
        w2pack[:, i * COUT : (i + 1) * COUT] = folded[p][1]
    w1pack[:, 4 * COUT : 5 * COUT] = np.eye(128, dtype=np.float32)
    in_maps = []
    for i in range(NCORES):
        m = {"x": np.ascontiguousarray(x[i * BP : (i + 1) * BP])}
        m["w1pack"] = w1pack.astype(bf)
        m["w2pack"] = w2pack.astype(bf)
        m["zc"] = np.zeros((CIN, 1), bf)
        m["onesrow"] = np.ones((1, L), bf)
        in_maps.append(m)
    return in_maps


def kernel(**inputs):
    nc = _get_nc()
    in_maps = make_in_maps(inputs)
    res = run_bass_kernel_spmd(nc, in_maps, core_ids=list(range(NCORES)))
    out = np.concatenate([res.results[i]["out"] for i in range(NCORES)], axis=0)
    return out.astype(np.float32)


if __name__ == "__main__":
    rng = np.random.default_rng(0)
    ins = {"x": rng.standard_normal((B, CIN, L), dtype=np.float32)}
    for p in ("q", "k", "v", "pe"):
        ins[f"{p}_w"] = (rng.standard_normal((COUT, CIN, KW)) * 0.05).astype(np.float32)
        ins[f"{p}_b"] = (rng.standard_normal(COUT) * 0.05).astype(np.float32)
        ins[f"{p}_gamma"] = rng.uniform(0.5, 1.5, COUT).astype(np.float32)
        ins[f"{p}_beta"] = (rng.standard_normal(COUT) * 0.05).astype(np.float32)
        ins[f"{p}_mean"] = (rng.standard_normal(COUT) * 0.05).astype(np.float32)
        ins[f"{p}_var"] = rng.uniform(0.5, 1.5, COUT).astype(np.float32)
    got = kernel(**ins)
    print("kernel output:", got.shape, got.dtype, np.abs(got).mean())
